# revision 19
# baseline (speedup 1.0000x reference)
"""Trainium2 Bass kernel for nn_ActionValues: conv stack + single LSTM step + heads.

Sharding (8 cores):
- w_ih gate-dim tensor-parallel: core j owns rows [576j,576j+576) of each of the
  i/g/o gates (f gate is dead since c0=0; w_hh unused since h0=0).
- conv1 contraction sharded (4 of 32 input channels per core) + AllReduce.
- conv2/3, small linears, heads replicated; head partials AllReduced.
Big matmuls run as float32r (TF32-like, ~1e-4 rel err).
"""
import sys

if "/opt/trn_rl_repo" not in sys.path:
    sys.path.insert(0, "/opt/trn_rl_repo")

from contextlib import ExitStack

import numpy as np

import concourse.mybir as mybir
import concourse.tile as tile
from concourse import bacc
from concourse.bass_utils import run_bass_kernel_spmd
from concourse.masks import make_identity

F32 = mybir.dt.float32
F32R = mybir.dt.float32r
AF = mybir.ActivationFunctionType

K = 12
MOA = 12
H = W = 24
D = 8
PHYS = MENT = 16
HSZ = 8 * H * W           # 4608
INP = 5024
EPS = 1e-5
MIX = 0.9
PIX = H * W               # 576
NPIX = K * PIX            # 6912
GS = HSZ // 8             # 576 gate-slice rows per core
RROWS = 3 * GS            # 1728 w rows per core (i,g,o slices)
NCC = 40                  # c-chunks: 39 x 128 + 1 x 32
N_EARLY = 8               # chunks whose transposes interleave with conv phase
SPANS = [(0, 432), (432, 432), (864, 432), (1296, 432)]

_NC_CACHE = {}


def r(ap):
    return ap


def build_graph(debug=False):
    nc = bacc.Bacc()
    P = {}
    for name, shape in [
        ("xs", [32, K, PIX]), ("wsl", [RROWS, INP]), ("bsl", [2, RROWS]),
        ("w1", [32, 16]), ("w2", [3, 48, 16]), ("w3", [5, 80, 8]),
        ("wpa", [17, 16]), ("wma", [17, 16]),
        ("pT1", [17, 12]), ("mT1", [17, 12]), ("mT9", [17, 12]),
        ("visT", [12, 12]),
        ("bn1", [16, 2]), ("bn2", [16, 2]), ("bn3", [8, 2]),
        ("ahw", [128, GS]), ("infw", [192, GS]), ("actw", [16, 128]),
        ("ahb", [1, 128]), ("infb", [1, 192]), ("actb", [1, 16]),
        ("zer", [1, 2048]),
    ]:
        dt = F32R if name in ("xs", "wsl", "bsl", "w1", "w2", "w3", "zer") else F32
        P[name] = nc.declare_dram_parameter(name, shape, dt, isOutput=False)
    out_ext = nc.declare_dram_parameter("out", [12, 208], F32, isOutput=True)
    dbg = nc.declare_dram_parameter("dbg", [128, 8864], F32, isOutput=True) if debug else None

    ar1_in = nc.dram_tensor("ar1_in", [16, NPIX], F32)
    ar1_out = nc.dram_tensor("ar1_out", [16, NPIX], F32)
    ar2_in = nc.dram_tensor("ar2_in", [12, 320], F32)
    ar2_out = nc.dram_tensor("ar2_out", [12, 320], F32)
    RG = [list(range(8))]

    cpctr = [0]

    def part_segs(p0, n):
        # legal compute-op partition segments: base 0 any, base 64 <=64, base 32/96 <=32
        out = []
        while n > 0:
            l = 128 if p0 == 0 else (32 if p0 % 64 == 32 else 64)
            l = min(l, n)
            out.append((p0, l))
            p0 += l
            n -= l
        return out

    def copy_any(dst, src):
        cpctr[0] += 1
        if cpctr[0] % 2 == 0:
            nc.vector.tensor_copy(dst, src)
        else:
            nc.scalar.copy(dst, src)

    with ExitStack() as ctx:
        tc = ctx.enter_context(tile.TileContext(nc))
        cp = ctx.enter_context(tc.tile_pool(name="const", bufs=1))
        convbuf = ctx.enter_context(tc.tile_pool(name="convbuf", bufs=2))
        wraw_pool = ctx.enter_context(tc.tile_pool(name="wraw", bufs=2))
        wt_pool = ctx.enter_context(tc.tile_pool(name="wt", bufs=N_EARLY))
        sb = ctx.enter_context(tc.tile_pool(name="sb", bufs=1))
        wt_ps = ctx.enter_context(tc.tile_pool(name="wt_ps", bufs=2, space="PSUM"))
        small_ps = ctx.enter_context(tc.tile_pool(name="small_ps", bufs=2, space="PSUM"))

        # ---------------- constants / tiny inputs ----------------
        ident = cp.tile([128, 128], F32)
        make_identity(nc, ident)
        identr = cp.tile([128, 128], F32R)
        nc.vector.tensor_copy(identr[:], ident[:])
        ones = cp.tile([128, 12], F32)
        nc.gpsimd.memset(ones[:], 1.0)
        ones_r = cp.tile([128, 12], F32R)
        nc.vector.tensor_copy(ones_r[:], ones[:])
        eps_t = cp.tile([16, 1], F32)
        nc.gpsimd.memset(eps_t[:], EPS)
        lstmT = cp.tile([128, 480], F32R)    # chunk cc at cols [12cc,12cc+12)

        t_in = {}
        for name, shape in [("w1", [32, 16]), ("wpa", [17, 16]), ("wma", [17, 16]),
                            ("pT1", [17, 12]), ("mT1", [17, 12]), ("mT9", [17, 12]),
                            ("bn1", [16, 2]), ("bn2", [16, 2]), ("bn3", [8, 2]),
                            ("actw", [16, 128])]:
            t = cp.tile(shape, F32R if name == "w1" else F32, tag=name)
            nc.gpsimd.dma_start(t[:], P[name].ap())
            t_in[name] = t
        w2t = cp.tile([48, 3 * 16], F32R)
        nc.gpsimd.dma_start(w2t[:].rearrange("p (a b) -> p a b", a=3),
                            P["w2"].ap().rearrange("a p b -> p a b"))
        w3t = cp.tile([80, 5 * 8], F32R)
        nc.gpsimd.dma_start(w3t[:].rearrange("p (a b) -> p a b", a=5),
                            P["w3"].ap().rearrange("a p b -> p a b"))
        btile = cp.tile([2, RROWS], F32R)
        nc.gpsimd.dma_start(btile[:], P["bsl"].ap())
        ahbB = cp.tile([12, 128], F32)
        nc.gpsimd.dma_start(ahbB[:], P["ahb"].ap().partition_broadcast(12))
        infbB = cp.tile([12, 192], F32)
        nc.gpsimd.dma_start(infbB[:], P["infb"].ap().partition_broadcast(12))
        actbB = cp.tile([12, 16], F32)
        nc.gpsimd.dma_start(actbB[:], P["actb"].ap().partition_broadcast(12))
        ahw = cp.tile([128, GS], F32)
        nc.sync.dma_start(ahw[:], P["ahw"].ap())
        infwA = cp.tile([128, GS], F32)
        nc.sync.dma_start(infwA[:], P["infw"].ap()[0:128, :])
        infwB = cp.tile([64, GS], F32)
        nc.sync.dma_start(infwB[:], P["infw"].ap()[128:192, :])
        xs = convbuf.tile([32, NPIX], F32R, tag="cbuf")
        nc.sync.dma_start(xs[:].rearrange("p (k q) -> p k q", k=K), P["xs"].ap())

        # W fetch: c-major [576,ksize] per gate; queue all fetches early on sync.
        wv = P["wsl"].ap().rearrange("(g rb q) c -> g rb q c", g=3, rb=6, q=96)
        wraw = []
        for cc in range(NCC):
            ks = 128 if cc < 39 else 32
            t = wraw_pool.tile([96, 3 * 6 * ks], F32R, tag="wraw")
            tv = t[:].rearrange("p (g rb c) -> p g rb c", g=3, rb=6)
            for g in range(3):
                nc.sync.dma_start(
                    tv[:, g],
                    wv[g, :, :, cc * 128:cc * 128 + ks].rearrange("rb q c -> q rb c"))
            wraw.append((t, ks))

        wt_tiles = {}

        def do_transposes(cc):
            t, ks = wraw[cc]
            tv = t[:].rearrange("p (g rb c) -> p g rb c", g=3, rb=6)
            wt = wt_pool.tile([128, RROWS], F32R, tag="wt")
            for b0 in range(0, 18, 5):
                nb = min(5, 18 - b0)
                pt = wt_ps.tile([128, 480], F32R, tag="wtp")
                for i in range(nb):
                    g, rb = divmod(b0 + i, 6)
                    nc.tensor.transpose(pt[0:ks, i * 96:(i + 1) * 96],
                                        tv[:, g, rb, :], identr[0:96, 0:96])
                copy_any(wt[0:ks, b0 * 96:(b0 + nb) * 96], pt[0:ks, 0:nb * 96])
            wt_tiles[cc] = (wt, ks)

        def do_mms(cc, gacc):
            wt, ks = wt_tiles.pop(cc)
            for s, (o, ln) in enumerate(SPANS):
                nc.tensor.matmul(gacc[s][:], r(lstmT[0:ks, cc * 12:cc * 12 + 12]),
                                 r(wt[0:ks, o:o + ln]),
                                 start=(cc == 0), stop=False, skip_group_check=True)

        # ---------------- tail assembly (lstmT chunks 36-39) ----------------
        pp = small_ps.tile([128, 512], F32, tag="sp")
        nc.tensor.matmul(pp[0:16, 0:12], r(t_in["wpa"][:]), r(t_in["pT1"][:]))
        nc.scalar.activation(lstmT[0:16, 432:444], pp[0:16, 0:12], AF.Relu)
        pm = small_ps.tile([128, 512], F32, tag="sp")
        nc.tensor.matmul(pm[0:16, 0:12], r(t_in["wma"][:]), r(t_in["mT1"][:]))
        mhat = cp.tile([16, 12], F32)
        nc.scalar.activation(mhat[:], pm[0:16, 0:12], AF.Relu)
        nc.gpsimd.dma_start(lstmT[16:32, 432:444], mhat[:])

        ps_p = small_ps.tile([128, 512], F32, tag="sp")
        nc.tensor.matmul(ps_p[0:12, 0:16], r(t_in["pT1"][:]), r(t_in["wpa"][:]))
        psb = cp.tile([12, 16], F32)
        nc.scalar.activation(psb[:], ps_p[0:12, 0:16], AF.Relu)
        ps_m = small_ps.tile([128, 512], F32, tag="sp")
        nc.tensor.matmul(ps_m[0:12, 0:16], r(t_in["mT9"][:]), r(t_in["wma"][:]))
        msb = cp.tile([12, 16], F32)
        nc.scalar.activation(msb[:], ps_m[0:12, 0:16], AF.Relu)

        vstage = cp.tile([128, 12], F32, tag="vs36")
        vstage7 = cp.tile([128, 12], F32, tag="vs37")
        for j in range(6):
            nc.gpsimd.dma_start(vstage[32 + 16 * j:48 + 16 * j, :],
                                P["visT"].ap()[j:j + 1, :].partition_broadcast(16))
            nc.gpsimd.dma_start(vstage7[16 * j:16 + 16 * j, :],
                                P["visT"].ap()[6 + j:7 + j, :].partition_broadcast(16))
        pf36 = cp.tile([128, 1], F32, tag="pf36")
        pf37 = cp.tile([128, 1], F32, tag="pf37")
        nc.gpsimd.dma_start(pf36[32:128, :], psb[0:6, :])
        nc.gpsimd.dma_start(pf37[0:96, :], psb[6:12, :])
        for q0, ql in part_segs(32, 96):
            nc.vector.tensor_scalar_mul(lstmT[q0:q0 + ql, 432:444],
                                        vstage[q0:q0 + ql, :], pf36[q0:q0 + ql, :])
        for q0, ql in part_segs(0, 96):
            nc.vector.tensor_scalar_mul(lstmT[q0:q0 + ql, 444:456],
                                        vstage7[q0:q0 + ql, :], pf37[q0:q0 + ql, :])
        mf37 = cp.tile([128, 1], F32, tag="mf37")
        mf38 = cp.tile([128, 1], F32, tag="mf38")
        mf39 = cp.tile([128, 1], F32, tag="mf39")
        nc.gpsimd.dma_start(mf37[96:128, :], msb[0:2, :])
        nc.gpsimd.dma_start(mf38[0:128, :], msb[2:10, :])
        nc.gpsimd.dma_start(mf39[0:32, :], msb[10:12, :])
        nc.vector.tensor_scalar_mul(lstmT[96:128, 444:456], ones[96:128, :], mf37[96:128, :])
        nc.vector.tensor_scalar_mul(lstmT[0:128, 456:468], ones[:], mf38[0:128, :])
        nc.vector.tensor_scalar_mul(lstmT[0:32, 468:480], ones[0:32, :], mf39[0:32, :])

        # ---------------- conv phase ----------------
        def bn_prep(src, nch, bn_t, tag):
            nst = (NPIX + 511) // 512
            stats = cp.tile([nch, nst * 6], F32, tag=tag + "_st")
            for i in range(nst):
                o = i * 512
                ln = min(512, NPIX - o)
                nc.vector.bn_stats(stats[:, i * 6:(i + 1) * 6], src[:, o:o + ln])
            mv = cp.tile([nch, 2], F32, tag=tag + "_mv")
            nc.vector.bn_aggr(mv[:], stats[:])
            sc = cp.tile([nch, 4], F32, tag=tag + "_sc")
            nc.scalar.activation(sc[:, 0:1], mv[:, 1:2], AF.Sqrt, bias=eps_t[0:nch, :])
            nc.vector.reciprocal(sc[:, 1:2], sc[:, 0:1])
            nc.vector.tensor_mul(sc[:, 2:3], sc[:, 1:2], bn_t[:, 0:1])
            nc.vector.tensor_mul(sc[:, 0:1], mv[:, 0:1], sc[:, 2:3])
            nc.vector.tensor_sub(sc[:, 3:4], bn_t[:, 1:2], sc[:, 0:1])
            return sc  # col2=scale', col3=shift'

        with tc.tile_pool(name="conv_ps", bufs=4, space="PSUM") as conv_ps:
            # conv1: [32,16].T x [32,6912]
            h1 = convbuf.tile([16, NPIX], F32, tag="cbuf")
            for o in range(0, NPIX, 512):
                ln = min(512, NPIX - o)
                pt = conv_ps.tile([16, 512], F32, tag="conv")
                nc.tensor.matmul(pt[:, 0:ln], r(t_in["w1"][:]), r(xs[:, o:o + ln]))
                copy_any(h1[:, o:o + ln], pt[:, 0:ln])
            nc.gpsimd.dma_start(ar1_in.ap(), h1[:])
            nc.gpsimd.collective_compute(
                "AllReduce", mybir.AluOpType.add, replica_groups=RG,
                ins=[ar1_in.ap()], outs=[ar1_out.ap()])
            nc.gpsimd.dma_start(h1[:], ar1_out.ap())

            for cc in range(0, 3):
                do_transposes(cc)

            sc1 = bn_prep(h1, 16, t_in["bn1"], "bn1")
            nc.scalar.activation(h1[:], h1[:], AF.Relu, bias=sc1[:, 3:4], scale=sc1[:, 2:3])
            if debug:
                nc.gpsimd.dma_start(dbg.ap()[0:16, 0:NPIX], h1[:])

            # B1: 3 dy-pre-shifted replicas of h1 padded to 26x26 (pad=1)
            B1 = convbuf.tile([48, K * 676], F32R, tag="cbuf")
            B1z = B1[:].rearrange("p (k q c) -> p k q c", k=K, q=26)
            for ap in (B1z[:, :, 0:2, :], B1z[:, :, 23:26, :],
                       B1z[:, :, :, 0:1], B1z[:, :, :, 25:26]):
                n = ap.free_size()
                nc.gpsimd.dma_start(ap, P["zer"].ap()[:, 0:n].partition_broadcast(48))
            h1v = h1[:].rearrange("p (k q r) -> p k q r", k=K, q=24)
            for g in range(3):
                B1v = B1[16 * g:16 * g + 16, :].rearrange("p (k q r) -> p k q r", k=K, q=26)
                rs, re = max(0, 1 - g), min(26, 25 - g)
                for k in range(K):
                    nc.gpsimd.dma_start(B1v[:, k, rs:re, 1:25],
                                        h1v[:, k, rs + g - 1:re + g - 1, :])
            B1f = B1[:].rearrange("p (k q r) -> p k q r", k=K, q=26)

            # conv2: accumulate 3 dx-taps, lhsT [48,16]
            h2 = convbuf.tile([16, NPIX], F32, tag="cbuf")
            w2v = w2t[:].rearrange("p (a b) -> p a b", a=3)
            for k in range(K):
                for r0 in (0, 12):
                    pt = conv_ps.tile([16, 512], F32, tag="conv")
                    for dx in range(3):
                        rhs = B1f[:, k, r0:r0 + 12, dx:dx + 24]
                        nc.tensor.matmul(pt[:, 0:288], r(w2v[:, dx]), r(rhs),
                                         start=(dx == 0), stop=(dx == 2),
                                         skip_group_check=True)
                    copy_any(h2[:, k * PIX + r0 * 24:k * PIX + (r0 + 12) * 24], pt[:, 0:288])
            for cc in range(3, 6):
                do_transposes(cc)
            sc2 = bn_prep(h2, 16, t_in["bn2"], "bn2")
            nc.scalar.activation(h2[:], h2[:], AF.Relu, bias=sc2[:, 3:4], scale=sc2[:, 2:3])

            # B2: 5 dy-pre-shifted replicas of h2 padded to 28x28 (pad=2)
            B2 = convbuf.tile([80, K * 784], F32R, tag="cbuf")
            B2z = B2[:].rearrange("p (k q c) -> p k q c", k=K, q=28)
            for ap in (B2z[:, :, 0:2, :], B2z[:, :, 22:28, :],
                       B2z[:, 0:6, :, 0:2], B2z[:, 6:12, :, 0:2],
                       B2z[:, 0:6, :, 26:28], B2z[:, 6:12, :, 26:28]):
                n = ap.free_size()
                nc.gpsimd.dma_start(ap, P["zer"].ap()[:, 0:n].partition_broadcast(80))
            h2v = h2[:].rearrange("p (k q r) -> p k q r", k=K, q=24)
            for g in range(5):
                B2v = B2[16 * g:16 * g + 16, :].rearrange("p (k q r) -> p k q r", k=K, q=28)
                rs, re = max(0, 2 - g), min(28, 26 - g)
                for k in range(K):
                    nc.gpsimd.dma_start(B2v[:, k, rs:re, 2:26],
                                        h2v[:, k, rs + g - 2:re + g - 2, :])
            B2f = B2[:].rearrange("p (k q r) -> p k q r", k=K, q=28)

            # conv3: accumulate 5 dx-taps, lhsT [80,8]
            h3 = convbuf.tile([8, NPIX], F32, tag="cbuf")
            w3v = w3t[:].rearrange("p (a b) -> p a b", a=5)
            for k in range(K):
                for r0 in (0, 12):
                    pt = conv_ps.tile([16, 512], F32, tag="conv")
                    for dx in range(5):
                        rhs = B2f[:, k, r0:r0 + 12, dx:dx + 24]
                        nc.tensor.matmul(pt[0:8, 0:288], r(w3v[:, dx]), r(rhs),
                                         start=(dx == 0), stop=(dx == 4),
                                         skip_group_check=True)
                    copy_any(h3[:, k * PIX + r0 * 24:k * PIX + (r0 + 12) * 24], pt[0:8, 0:288])
            for cc in range(6, N_EARLY):
                do_transposes(cc)

            # bn3 stats on [8,NPIX]; normalize+relu on stacked [96,576] layout
            sc3 = bn_prep(h3, 8, t_in["bn3"], "bn3")
            h3s = cp.tile([96, PIX], F32)
            for k in range(K):
                nc.gpsimd.dma_start(h3s[8 * k:8 * k + 8, :], h3[:, k * PIX:(k + 1) * PIX])
            sc3s = cp.tile([96, 2], F32)
            for k in range(K):
                nc.gpsimd.dma_start(sc3s[8 * k:8 * k + 8, :], sc3[:, 2:4])
            nc.scalar.activation(h3s[:], h3s[:], AF.Relu, bias=sc3s[:, 1:2], scale=sc3s[:, 0:1])
            if debug:
                nc.gpsimd.dma_start(dbg.ap()[0:96, 7392:7968], h3s[:])

            # h3T: 6 transposes [96,96] -> strided copies into lstmT chunks 0..35
            for a0 in range(0, 6, 5):
                nb = min(5, 6 - a0)
                pt = small_ps.tile([128, 512], F32, tag="sp")
                for i in range(nb):
                    nc.tensor.transpose(r(pt[0:96, i * 96:(i + 1) * 96]),
                                        r(h3s[:, (a0 + i) * 96:(a0 + i + 1) * 96]),
                                        r(ident[0:96, 0:96]))
                for i in range(nb):
                    srcv = pt[0:96, i * 96:(i + 1) * 96].rearrange("p (k c) -> p k c", k=K)
                    for c3 in range(8):
                        rr = c3 * 576 + (a0 + i) * 96
                        t0, p0 = divmod(rr, 128)
                        n1 = min(96, 128 - p0)
                        for q0 in range(0, n1, 32):
                            copy_any(lstmT[p0 + q0:p0 + q0 + 32, t0 * 12:t0 * 12 + 12],
                                     srcv[q0:q0 + 32, :, c3])
                        for q0 in range(0, 96 - n1, 32):
                            copy_any(lstmT[q0:q0 + 32,
                                           (t0 + 1) * 12:(t0 + 1) * 12 + 12],
                                     srcv[n1 + q0:n1 + q0 + 32, :, c3])

        # ---------------- gates stream ----------------
        with tc.tile_pool(name="gates_ps", bufs=1, space="PSUM") as gates_ps:
            gacc = [gates_ps.tile([12, 432], F32, tag=f"g{s}", name=f"gacc{s}")
                    for s in range(4)]
            if debug:
                nc.gpsimd.dma_start(dbg.ap()[0:128, 6912:7392], lstmT[:].bitcast(F32))
            for cc in range(0, N_EARLY):
                do_mms(cc, gacc)
            for cc in range(N_EARLY, NCC):
                do_transposes(cc)
                do_mms(cc, gacc)
            for s, (o, ln) in enumerate(SPANS):
                nc.tensor.matmul(gacc[s][:], ones_r[0:2, :], btile[0:2, o:o + ln],
                                 start=False, stop=True, skip_group_check=True)

            # gate nonlinearities: R-layout [i(576) g(576) o(576)] over 4 spans
            sig_i = sb.tile([12, GS], F32, tag="gi")
            tan_g = sb.tile([12, GS], F32, tag="gg")
            sig_o = sb.tile([12, GS], F32, tag="go")

            def gate_act(lo, hi, fn, dst):
                base = lo
                while lo < hi:
                    s = lo // 432
                    o = lo - s * 432
                    ln = min(hi - lo, 432 - o)
                    nc.scalar.activation(dst[:, lo - base:lo - base + ln],
                                         gacc[s][:, o:o + ln], fn)
                    lo += ln

            gate_act(0, 576, AF.Sigmoid, sig_i)
            gate_act(576, 1152, AF.Tanh, tan_g)
            gate_act(1152, 1728, AF.Sigmoid, sig_o)
            hr = sb.tile([12, GS], F32, tag="hr")
            nc.vector.tensor_mul(hr[:], sig_i[:], tan_g[:])
            nc.scalar.activation(hr[:], hr[:], AF.Tanh)
            nc.vector.tensor_mul(hr[:], sig_o[:], hr[:])
            nc.scalar.activation(hr[:], hr[:], AF.Relu)
            if debug:
                nc.gpsimd.dma_start(dbg.ap()[0:12, 7968:8544], hr[:])

            # hrT [128,60]: transposes of hr 128-chunks (last=64)
            pt = small_ps.tile([128, 512], F32, tag="sp")
            for n in range(5):
                ks = 128 if n < 4 else 64
                nc.tensor.transpose(r(pt[0:ks, n * 12:(n + 1) * 12]),
                                    r(hr[:, n * 128:n * 128 + ks]), r(ident[0:12, 0:12]))
            hrT = cp.tile([128, 60], F32)
            nc.vector.tensor_copy(hrT[:], pt[:, 0:60])

            # ahwT [128,640] (chunk n at cols 128n) / infwT [128,960] (chunk n at 192n)
            ahwT = cp.tile([128, 640], F32)
            infwT = cp.tile([128, 960], F32)
            pta = small_ps.tile([128, 512], F32, tag="sp")
            for n in range(4):
                nc.tensor.transpose(r(pta[:, n * 128:(n + 1) * 128]),
                                    r(ahw[:, n * 128:(n + 1) * 128]), r(ident[:]))
            nc.scalar.copy(ahwT[:, 0:512], pta[:])
            ptb = small_ps.tile([128, 512], F32, tag="sp")
            nc.tensor.transpose(r(ptb[0:64, 0:128]), r(ahw[:, 512:576]), r(ident[:]))
            nc.tensor.transpose(r(ptb[:, 128:256]), r(infwA[:, 0:128]), r(ident[:]))
            nc.tensor.transpose(r(ptb[:, 256:384]), r(infwA[:, 128:256]), r(ident[:]))
            nc.tensor.transpose(r(ptb[:, 384:448]), r(infwB[:, 0:128]), r(ident[0:64, 0:64]))
            nc.scalar.copy(ahwT[0:64, 512:640], ptb[0:64, 0:128])
            nc.scalar.copy(infwT[:, 0:128], ptb[:, 128:256])
            nc.scalar.copy(infwT[:, 192:320], ptb[:, 256:384])
            nc.scalar.copy(infwT[:, 128:192], ptb[:, 384:448])
            ptc = small_ps.tile([128, 512], F32, tag="sp")
            nc.tensor.transpose(r(ptc[:, 0:128]), r(infwA[:, 256:384]), r(ident[:]))
            nc.tensor.transpose(r(ptc[:, 128:256]), r(infwA[:, 384:512]), r(ident[:]))
            nc.tensor.transpose(r(ptc[0:64, 256:384]), r(infwA[:, 512:576]), r(ident[:]))
            nc.tensor.transpose(r(ptc[:, 384:448]), r(infwB[:, 128:256]), r(ident[0:64, 0:64]))
            nc.scalar.copy(infwT[:, 384:512], ptc[:, 0:128])
            nc.scalar.copy(infwT[:, 576:704], ptc[:, 128:256])
            nc.scalar.copy(infwT[0:64, 768:896], ptc[0:64, 256:384])
            nc.scalar.copy(infwT[:, 320:384], ptc[:, 384:448])
            ptd = small_ps.tile([128, 512], F32, tag="sp")
            nc.tensor.transpose(r(ptd[:, 0:64]), r(infwB[:, 256:384]), r(ident[0:64, 0:64]))
            nc.tensor.transpose(r(ptd[:, 64:128]), r(infwB[:, 384:512]), r(ident[0:64, 0:64]))
            nc.tensor.transpose(r(ptd[0:64, 128:192]), r(infwB[:, 512:576]), r(ident[0:64, 0:64]))
            nc.scalar.copy(infwT[:, 512:576], ptd[:, 0:64])
            nc.scalar.copy(infwT[:, 704:768], ptd[:, 64:128])
            nc.scalar.copy(infwT[0:64, 896:960], ptd[0:64, 128:192])

            # head partial matmuls, AllReduce, final heads
            pah = small_ps.tile([128, 512], F32, tag="sp")
            for n in range(5):
                ks = 128 if n < 4 else 64
                nc.tensor.matmul(pah[0:12, 0:128], r(hrT[0:ks, n * 12:(n + 1) * 12]),
                                 r(ahwT[0:ks, n * 128:(n + 1) * 128]),
                                 start=(n == 0), stop=(n == 4), skip_group_check=True)
            pinf = small_ps.tile([128, 512], F32, tag="sp")
            for n in range(5):
                ks = 128 if n < 4 else 64
                nc.tensor.matmul(pinf[0:12, 0:192], r(hrT[0:ks, n * 12:(n + 1) * 12]),
                                 r(infwT[0:ks, n * 192:(n + 1) * 192]),
                                 start=(n == 0), stop=(n == 4), skip_group_check=True)
            s2 = sb.tile([12, 320], F32, tag="s2")
            nc.vector.tensor_copy(s2[:, 0:128], pah[0:12, 0:128])
            nc.vector.tensor_copy(s2[:, 128:320], pinf[0:12, 0:192])
            if debug:
                nc.gpsimd.dma_start(dbg.ap()[0:12, 8544:8864], s2[:])
            nc.gpsimd.dma_start(ar2_in.ap(), s2[:])
            nc.gpsimd.collective_compute(
                "AllReduce", mybir.AluOpType.add, replica_groups=RG,
                ins=[ar2_in.ap()], outs=[ar2_out.ap()])
            s2r = sb.tile([12, 320], F32, tag="s2r")
            nc.gpsimd.dma_start(s2r[:], ar2_out.ap())

            out_sb = sb.tile([12, 208], F32, tag="osb")
            nc.vector.tensor_add(out_sb[:, 16:208], s2r[:, 128:320], infbB[:])
            a2 = sb.tile([12, 128], F32, tag="a2")
            nc.vector.tensor_add(a2[:], s2r[:, 0:128], ahbB[:])
            nc.scalar.activation(a2[:], a2[:], AF.Relu)
            pte = small_ps.tile([128, 512], F32, tag="sp")
            nc.tensor.transpose(r(pte[:, 0:12]), r(a2[:]), r(ident[0:12, 0:12]))
            nc.tensor.transpose(r(pte[:, 12:28]), r(t_in["actw"][:]), r(ident[0:16, 0:16]))
            a2T = cp.tile([128, 28], F32)
            nc.vector.tensor_copy(a2T[:], pte[:, 0:28])
            ptf = small_ps.tile([128, 512], F32, tag="sp")
            nc.tensor.matmul(ptf[0:12, 0:16], r(a2T[:, 0:12]), r(a2T[:, 12:28]))
            nc.vector.tensor_add(out_sb[:, 0:16], ptf[0:12, 0:16], actbB[:])
            nc.sync.dma_start(out_ext.ap(), out_sb[:])

    nc.finalize()
    return nc


def shard_inputs(inputs):
    f = lambda a: np.ascontiguousarray(np.asarray(a, dtype=np.float32))
    x = f(inputs["x"])
    w_ih = f(inputs["w_ih"])
    b_ih, b_hh = f(inputs["b_ih"]), f(inputs["b_hh"])
    p, m, vis = f(inputs["p"]), f(inputs["m"]), f(inputs["vis"])
    phys_w, phys_b = f(inputs["phys_w"]), f(inputs["phys_b"])
    ment_w, ment_b = f(inputs["ment_w"]), f(inputs["ment_b"])
    ones = np.ones((1, 12), np.float32)
    shared = {
        "wpa": np.concatenate([phys_w.T, phys_b[None, :]], 0),
        "wma": np.concatenate([ment_w.T, ment_b[None, :]], 0),
        "pT1": np.concatenate([p.T, ones], 0),
        "mT1": np.concatenate([m.T, ones], 0),
        "mT9": np.concatenate([MIX * m.T, ones], 0),
        "w2": np.ascontiguousarray(
            np.transpose(f(inputs["conv2_w"]), (3, 2, 1, 0)).reshape(3, 48, 16)),
        "w3": np.ascontiguousarray(
            np.transpose(f(inputs["conv3_w"]), (3, 2, 1, 0)).reshape(5, 80, 8)),
        "bn1": np.ascontiguousarray(np.stack([f(inputs["bn1_g"]), f(inputs["bn1_b"])], 1)),
        "bn2": np.ascontiguousarray(np.stack([f(inputs["bn2_g"]), f(inputs["bn2_b"])], 1)),
        "bn3": np.ascontiguousarray(np.stack([f(inputs["bn3_g"]), f(inputs["bn3_b"])], 1)),
        "visT": np.ascontiguousarray(vis.T),
        "actw": f(inputs["act_w"]),
        "ahb": f(inputs["ah_b"])[None, :],
        "infb": f(inputs["inf_b"])[None, :],
        "actb": f(inputs["act_b"])[None, :],
        "zer": np.zeros((1, 2048), np.float32),
    }
    ah_w, inf_w = f(inputs["ah_w"]), f(inputs["inf_w"])
    conv1_w = f(inputs["conv1_w"])
    in_maps = []
    for j in range(8):
        sl = slice(GS * j, GS * (j + 1))
        idx = np.arange(GS * j, GS * (j + 1))
        rows = np.concatenate([idx, 2 * HSZ + idx, 3 * HSZ + idx])
        d = dict(shared)
        d.update({
            "xs": np.ascontiguousarray(
                x[:, 4 * j:4 * j + 4].reshape(K, 32, PIX).transpose(1, 0, 2)),
            "wsl": np.ascontiguousarray(w_ih[rows]),
            "bsl": np.ascontiguousarray(np.stack([b_ih[rows], b_hh[rows]], 0)),
            "w1": np.ascontiguousarray(
                conv1_w[:, 4 * j:4 * j + 4, :].reshape(16, 32).T),
            "ahw": np.ascontiguousarray(ah_w[:, sl]),
            "infw": np.ascontiguousarray(inf_w[:, sl]),
        })
        in_maps.append(d)
    return in_maps


def _get_nc(debug=False):
    key = ("nc", debug)
    if key not in _NC_CACHE:
        _NC_CACHE[key] = build_graph(debug)
    return _NC_CACHE[key]


def run(inputs, trace=False, debug=False):
    nc = _get_nc(debug)
    in_maps = shard_inputs(inputs)
    res = run_bass_kernel_spmd(nc, in_maps, core_ids=list(range(8)), trace=trace)
    out = res.results[0]["out"]
    act = np.ascontiguousarray(out[:, 0:16])
    old_ment = np.ascontiguousarray(out[:, 16:208]).reshape(K, MOA, MENT)
    return (act, old_ment), res


def kernel(**inputs):
    (act, old_ment), _ = run(inputs, trace=False)
    return act, old_ment
